# revision 2
# baseline (speedup 1.0000x reference)
"""Bass/Trainium2 kernel for nn_BitwiseTasNetRepeat.

Strategy
--------
Every sign(BN(.)) in the reference collapses to a per-channel threshold
compare (BN gamma > 0), so the whole block chain becomes:

    S1 = sign(R - thr1)                    (ACT Sign, bias = -thr1)
    p1 = sign(w1) @ S1                     (TensorE, bf16 exact: operands +-1)
    S2 = sign(p1 - thr2)                   (ACT Sign from PSUM)
    q  = a0*S2(-d) + S2(0) + a2*S2(+d)     (DVE scalar_tensor_tensor x2)
    S3 = (q >= ctr*thr3) - 0.5             (DVE tensor_scalar is_ge/sub)
    p2 = (sign(w2)*ctr) @ S3               (TensorE)
    R += 2*p2                              (DVE scalar_tensor_tensor)

where d = 2^i, ctr = sign(dw_w[:,1]), a0 = sign(dw_w[:,0])*ctr,
a2 = sign(dw_w[:,2])*ctr.  The center-tap sign ctr is folded into the
conv2 weights and thr3; the 0.5 amplitude of S3 is folded into the
"*2" of the residual update.  All arithmetic is exact in fp32/bf16
(matmul operands are +-1, accumulation in fp32 PSUM).

Sharding: data-parallel over batch, 2 batches per core, 8 cores.
"""

import numpy as np
import ml_dtypes

_B, _CB, _H, _T = 16, 256, 512, 4096
_BLOCKS = 8
_EPS = 1e-5
_NCORES = 8
_BS = _B // _NCORES      # batches per core
_KC = _CB // 128         # 2  k-tiles of Cb
_MH = _H // 128          # 4  m-tiles of H
_NT = _T // 512          # 8  n-tiles of T
_PAD = 128               # halo for dilated depthwise conv (max d = 128)
_NCC = 18                # const columns per block

_nc_cache = {}


def _build_nc(bs=_BS, nblocks=_BLOCKS, T=_T):
    import concourse.bass as bass
    import concourse.mybir as mybir
    from concourse import bacc
    from concourse.tile import TileContext

    f32 = mybir.dt.float32
    bf16 = mybir.dt.bfloat16
    ALU = mybir.AluOpType
    nt = T // 512

    nc = bacc.Bacc("TRN2", target_bir_lowering=False, debug=False,
                   enable_asserts=False)

    x_d = nc.dram_tensor("x", [bs, _CB, T], f32, kind="ExternalInput")
    w1_d = nc.dram_tensor("w1sb", [128, nblocks * _KC * _MH * 128], bf16,
                          kind="ExternalInput")
    w2_d = nc.dram_tensor("w2sb", [128, nblocks * _MH * _KC * 128], bf16,
                          kind="ExternalInput")
    cst_d = nc.dram_tensor("cst", [128, nblocks * _NCC], f32,
                           kind="ExternalInput")
    out_d = nc.dram_tensor("out", [bs, _CB, T], f32, kind="ExternalOutput")

    with TileContext(nc) as tc:
        with (
            tc.tile_pool(name="wpool", bufs=1) as wpool,
            tc.tile_pool(name="cpool", bufs=1) as cpool,
            tc.tile_pool(name="rpool", bufs=2) as rpool,
            tc.tile_pool(name="s1pool", bufs=3) as s1pool,
            tc.tile_pool(name="s2pool", bufs=5) as s2pool,
            tc.tile_pool(name="s3pool", bufs=4) as s3pool,
            tc.tile_pool(name="qpool", bufs=2) as qpool,
            tc.tile_pool(name="psA", bufs=4, space="PSUM") as psA,
            tc.tile_pool(name="psC", bufs=4, space="PSUM") as psC,
        ):
            w1sb = wpool.tile([128, nblocks * _KC * _MH * 128], bf16)
            nc.sync.dma_start(out=w1sb[:], in_=w1_d.ap())
            w2sb = wpool.tile([128, nblocks * _MH * _KC * 128], bf16)
            nc.sync.dma_start(out=w2sb[:], in_=w2_d.ap())
            cst = cpool.tile([128, nblocks * _NCC], f32)
            nc.sync.dma_start(out=cst[:], in_=cst_d.ap())

            def w1t(i, kc, mh):
                o = (i * _KC * _MH + kc * _MH + mh) * 128
                return w1sb[:, o:o + 128]

            def w2t(i, kh, mc):
                o = (i * _MH * _KC + kh * _KC + mc) * 128
                return w2sb[:, o:o + 128]

            def cc(i, j):
                return cst[:, i * _NCC + j:i * _NCC + j + 1]

            for b in range(bs):
                R = []
                for kc in range(_KC):
                    rt = rpool.tile([128, T], f32, tag="R",
                                    name=f"R_b{b}_{kc}")
                    nc.sync.dma_start(out=rt[:], in_=x_d.ap()[b, kc * 128:(kc + 1) * 128, :])
                    R.append(rt)

                for i in range(nblocks):
                    d = 2 ** i
                    # ---- stage A: u1 threshold + conv1 + u2 threshold ----
                    S1 = []
                    for kc in range(_KC):
                        s1t = s1pool.tile([128, T], bf16, tag="S1",
                                          name=f"S1_b{b}_i{i}_{kc}")
                        nc.scalar.sign(s1t[:], R[kc][:], bias=cc(i, kc))
                        S1.append(s1t)
                    S2 = []
                    for mh in range(_MH):
                        s2t = s2pool.tile([128, T + 2 * _PAD], bf16, tag="S2",
                                          name=f"S2_b{b}_i{i}_{mh}")
                        nc.vector.memset(s2t[:, 0:_PAD], 0.0)
                        nc.vector.memset(s2t[:, _PAD + T:2 * _PAD + T], 0.0)
                        S2.append(s2t)
                    for mh in range(_MH):
                        for n in range(nt):
                            ps = psA.tile([128, 512], mybir.dt.float32,
                                          tag="psA", name=f"psA_{b}_{i}_{mh}_{n}")
                            for kc in range(_KC):
                                nc.tensor.matmul(
                                    ps[:], w1t(i, kc, mh),
                                    S1[kc][:, n * 512:(n + 1) * 512],
                                    start=(kc == 0), stop=(kc == _KC - 1))
                            nc.scalar.sign(
                                S2[mh][:, _PAD + n * 512:_PAD + (n + 1) * 512],
                                ps[:], bias=cc(i, 2 + mh))
                    # ---- stage B: depthwise dilated conv + u3 threshold ----
                    S3 = []
                    for mh in range(_MH):
                        tmp = qpool.tile([128, T], bf16, tag="tmp",
                                         name=f"tmp_b{b}_i{i}_{mh}")
                        nc.vector.scalar_tensor_tensor(
                            tmp[:], S2[mh][:, _PAD - d:_PAD - d + T],
                            cc(i, 10 + mh), S2[mh][:, _PAD:_PAD + T],
                            op0=ALU.mult, op1=ALU.add)
                        q = qpool.tile([128, T], bf16, tag="q",
                                       name=f"q_b{b}_i{i}_{mh}")
                        nc.vector.scalar_tensor_tensor(
                            q[:], S2[mh][:, _PAD + d:_PAD + d + T],
                            cc(i, 14 + mh), tmp[:],
                            op0=ALU.mult, op1=ALU.add)
                        s3t = s3pool.tile([128, T], bf16, tag="S3",
                                          name=f"S3_b{b}_i{i}_{mh}")
                        nc.vector.tensor_scalar(
                            s3t[:], q[:], cc(i, 6 + mh), 0.5,
                            op0=ALU.is_ge, op1=ALU.subtract)
                        S3.append(s3t)
                    # ---- stage C: conv2 + residual update ----
                    for mc in range(_KC):
                        for n in range(nt):
                            ps2 = psC.tile([128, 512], mybir.dt.float32,
                                           tag="psC", name=f"psC_{b}_{i}_{mc}_{n}")
                            for kh in range(_MH):
                                nc.tensor.matmul(
                                    ps2[:], w2t(i, kh, mc),
                                    S3[kh][:, n * 512:(n + 1) * 512],
                                    start=(kh == 0), stop=(kh == _MH - 1))
                            nc.vector.scalar_tensor_tensor(
                                R[mc][:, n * 512:(n + 1) * 512], ps2[:], 2.0,
                                R[mc][:, n * 512:(n + 1) * 512],
                                op0=ALU.mult, op1=ALU.add)

                for kc in range(_KC):
                    nc.sync.dma_start(out=out_d.ap()[b, kc * 128:(kc + 1) * 128, :],
                                      in_=R[kc][:])
    nc.finalize()
    return nc


def _prep(inputs, nblocks=_BLOCKS):
    """Host-side weight/threshold preprocessing (tiny tensors only)."""
    def thr(g, bb, m, v):
        return (m - bb * np.sqrt(v + _EPS) / g).astype(np.float32)

    w1sb = np.zeros((128, nblocks * _KC * _MH * 128), ml_dtypes.bfloat16)
    w2sb = np.zeros((128, nblocks * _MH * _KC * 128), ml_dtypes.bfloat16)
    cst = np.zeros((128, nblocks * _NCC), np.float32)
    for i in range(nblocks):
        t1 = thr(inputs['bn1_gamma'][i], inputs['bn1_beta'][i],
                 inputs['bn1_mean'][i], inputs['bn1_var'][i])          # [Cb]
        t2 = thr(inputs['bn2_gamma'][i], inputs['bn2_beta'][i],
                 inputs['bn2_mean'][i], inputs['bn2_var'][i])          # [H]
        t3 = thr(inputs['bn3_gamma'][i], inputs['bn3_beta'][i],
                 inputs['bn3_mean'][i], inputs['bn3_var'][i])          # [H]
        W1s = np.sign(inputs['w1'][i]).astype(np.float32)              # [H, Cb]
        W2s = np.sign(inputs['w2'][i]).astype(np.float32)              # [Cb, H]
        dws = np.sign(inputs['dw_w'][i]).astype(np.float32)            # [H, 3]
        ctr = dws[:, 1]
        a0 = dws[:, 0] * ctr
        a2 = dws[:, 2] * ctr
        W2p = W2s * ctr[None, :]                                       # [Cb, H]
        for kc in range(_KC):
            for mh in range(_MH):
                o = (i * _KC * _MH + kc * _MH + mh) * 128
                # lhsT1[p, f] = W1s[mh*128+f, kc*128+p]
                w1sb[:, o:o + 128] = W1s[mh * 128:(mh + 1) * 128,
                                         kc * 128:(kc + 1) * 128].T
        for kh in range(_MH):
            for mc in range(_KC):
                o = (i * _MH * _KC + kh * _KC + mc) * 128
                # lhsT2[p, f] = W2p[mc*128+f, kh*128+p]
                w2sb[:, o:o + 128] = W2p[mc * 128:(mc + 1) * 128,
                                         kh * 128:(kh + 1) * 128].T
        base = i * _NCC
        for kc in range(_KC):
            cst[:, base + kc] = -t1[kc * 128:(kc + 1) * 128]
        for mh in range(_MH):
            sl = slice(mh * 128, (mh + 1) * 128)
            cst[:, base + 2 + mh] = -t2[sl]
            cst[:, base + 6 + mh] = (ctr * t3)[sl]
            cst[:, base + 10 + mh] = a0[sl]
            cst[:, base + 14 + mh] = a2[sl]
    return w1sb, w2sb, cst


def kernel(**inputs):
    inputs = {k: np.asarray(v) for k, v in inputs.items()}
    x = inputs['x'].astype(np.float32)
    w1sb, w2sb, cst = _prep(inputs)

    if 'nc' not in _nc_cache:
        _nc_cache['nc'] = _build_nc()
    nc = _nc_cache['nc']

    in_maps = []
    for c in range(_NCORES):
        in_maps.append({
            'x': np.ascontiguousarray(x[c * _BS:(c + 1) * _BS]),
            'w1sb': w1sb, 'w2sb': w2sb, 'cst': cst,
        })

    from concourse.bass_utils import run_bass_kernel_spmd
    import os
    trace = bool(int(os.environ.get('KERNEL_TRACE', '0')))
    res = run_bass_kernel_spmd(nc, in_maps, core_ids=list(range(_NCORES)),
                               trace=trace)
    _nc_cache['last_result'] = res
    out = np.concatenate([r['out'] for r in res.results], axis=0)
    return out.astype(np.float32)
